# revision 50
# baseline (speedup 1.0000x reference)
"""Causal self-attention (RoPE) Trainium2 Bass kernel (v3).

Sharding: 8 cores = 2 (batch) x 4 (head groups). Each core computes one batch
element b and 4 of the 16 heads end-to-end (QKV projection -> RoPE -> causal
attention -> c_proj rows), producing a partial [T, C] output; the host sums
the 4 partials per batch element (the "all-reduce" of the row-sharded c_proj).

v2 layout tricks (kept):
- Q/K are computed head-major with lo/hi RoPE halves INTERLEAVED pairwise
  (row 2d = dim d, row 2d+1 = dim d+32 of a head). rotate_half is then a
  within-32-partition swap = one DVE stream_shuffle, and each head occupies
  64 contiguous partitions so the scores matmul contracts 64 rows in ONE
  instruction. The host permutes W_attn's Q/K columns to emit this layout.
- bf16 operands everywhere on the PE.
- Diagonal 128-col blocks restrict exp/AV to the unmasked query range.
- qkv-projection and c_proj units of neighboring tiles are interleaved into
  the attention kb loop so the PE fills the exp-latency gaps.

v3.x changes (trace-driven, from v2 @184us -> ~169us):
- AV matmuls software-pipelined PIPE=3 kb-iterations behind their exp, so
  the in-order PE queue never stalls on the ACT engine (exp is the
  throughput bound of late attention tiles).
- q/k projection in fp8e4 DoubleRow (contracts 256 rows/pass, 2x): W
  pre-scaled x16 to clear the e4m3 subnormal range, un-scale folded into
  exp's scale argument.  V and c_proj stay bf16 (v/y quantization error
  would land directly in the output; q/k noise averages out in softmax).
- Warm-up matmuls on memset tiles at t=0 and dummy warm matmuls inside
  exp-bound attention tiles + the tail keep the PE HAM clock gate at 8/8
  (idle stretches re-throttle the PE to 1.2GHz for >=3.4us).
- Each tile's V projections run as EARLY fillers of its OWN attention tile
  (AV consumes v_aug[kb] only at iteration kb+PIPE+1), so the PE is not
  queue-blocked on the 2MB bf16 xt0 DMA; q/k fillers go early (rope chains
  must drain before the next tile), cproj fillers spread late.
- DMA: startup tensors split per contraction chunk across the SP and ACT
  HWDGE queues (descriptor issue is ~600ns serial per queue; transfers are
  packet-split over 16 engines regardless), later tensors one descriptor.
- PSUM evictions (rope p_s, v_aug, grp-0 y rows) on the scalar engine for
  early tiles where ACT has slack and DVE is the bottleneck; DVE keeps
  them in the exp-bound tiles.
- Output stored as bf16 (halves out DMA; host sums partials in f32), one
  descriptor per 128-token block (two in the drain-critical tail).
- 1/sumexp chain batched: both heads' rows staged at partitions 0/32, one
  reciprocal instruction (reciprocal_approx_fast must NOT read PSUM
  directly -- silent garbage on HW).
"""

import os
import sys
import numpy as np

N_CORES = 8
B, T, C = 2, 2048, 1024
H = 16
HD = 64
HPC = 4            # heads per core
NT = 4             # token tiles of 512
TQ = 512           # tq tile size
KC = C // 128      # contraction chunks for qkv projection
PIPE = 3           # AV trails exp by this many kb iterations
N_WARM = 10        # HAM warm-up matmuls
OUT_BF16 = True
USE_FP8_QK = True  # q/k projection via fp8e4 DoubleRow (W pre-scaled x16)
QK_W_SCALE = 16.0
KC2 = C // 256     # DoubleRow contraction chunks (256 rows each)

_PROGRAM_CACHE = {}

# stream_shuffle mask: swap even/odd partitions within each 32-block
SWAP_EVEN_ODD = [i ^ 1 for i in range(32)]


def _build_program(has_battn: bool, has_bproj: bool):
    import concourse.bass as bass
    import concourse.mybir as mybir
    import concourse.bacc as bacc
    import concourse.tile as tile

    F32 = mybir.dt.float32
    F32R = mybir.dt.float32r
    BF16 = mybir.dt.bfloat16
    FP8 = mybir.dt.float8e4
    OUT_DT = BF16 if OUT_BF16 else F32

    nc = bacc.Bacc("TRN2", target_bir_lowering=False, debug=False,
                   num_devices=N_CORES)

    xT = nc.dram_tensor("xT", [C, T], BF16, kind="ExternalInput").ap()
    if USE_FP8_QK:
        # fp8 copies of x and the (16x pre-scaled, DoubleRow-interleaved)
        # q/k weight columns; the 1/16^2 un-scale is folded into exp's scale
        xT8 = nc.dram_tensor("xT8", [C, T], FP8, kind="ExternalInput").ap()
        wqk8 = nc.dram_tensor("wqk8", [C, 512], FP8, kind="ExternalInput").ap()
    wqkv = nc.dram_tensor("wqkv", [C, 768], BF16, kind="ExternalInput").ap()
    cos_il = nc.dram_tensor("cos_il", [128, T], BF16, kind="ExternalInput").ap()
    sin_il = nc.dram_tensor("sin_il", [128, T], BF16, kind="ExternalInput").ap()
    wp = nc.dram_tensor("wp", [2 * 128, C], BF16, kind="ExternalInput").ap()
    battn = (nc.dram_tensor("battn", [1, 768], BF16, kind="ExternalInput").ap()
             if has_battn else None)
    bproj = (nc.dram_tensor("bproj", [1, C], F32R, kind="ExternalInput").ap()
             if has_bproj else None)
    out = nc.dram_tensor("out", [T, C], OUT_DT, kind="ExternalOutput").ap()

    Exp = mybir.ActivationFunctionType.Exp
    scale = 1.0 / float(np.sqrt(HD))
    if USE_FP8_QK:
        scale /= QK_W_SCALE * QK_W_SCALE
    DR = mybir.MatmulPerfMode.DoubleRow

    with tile.TileContext(nc) as tc:
        with (
            tc.tile_pool(name="const", bufs=1) as const,
            tc.tile_pool(name="xp", bufs=1) as xp,
            tc.tile_pool(name="qk", bufs=1) as qkp,
            tc.tile_pool(name="vaug", bufs=1) as vaugp,
            tc.tile_pool(name="tmp", bufs=4) as tmp,
            tc.tile_pool(name="shp", bufs=3) as shp,
            tc.tile_pool(name="pp", bufs=5) as pp,
            tc.tile_pool(name="rp", bufs=4) as rp,
            tc.tile_pool(name="yrp", bufs=4) as yrp,
            tc.tile_pool(name="yp", bufs=1) as yp,
            tc.tile_pool(name="op", bufs=3) as op,
            tc.tile_pool(name="ps_qv", bufs=2, space="PSUM") as ps_qv,
            tc.tile_pool(name="ps_s", bufs=2, space="PSUM") as ps_s,
            tc.tile_pool(name="ps_av", bufs=2, space="PSUM") as ps_av,
        ):
            # ---- input DMAs.  The critical first-tile data (w + xt0) is
            # split per-kc chunk across BOTH HWDGE queues (SP gets w, ACT
            # gets xt0) so the qkv(0) accumulation chain can advance chunk
            # by chunk as data lands; later tensors use one descriptor each
            # (the HW packet-splitter spreads any descriptor over 16 DMA
            # engines, so descriptor count only costs serial issue time).
            cos_sb = const.tile([128, T], BF16, tag="cos")
            sin_sb = const.tile([128, T], BF16, tag="sin")
            xt_view = xT.rearrange("(kc p) t -> p kc t", p=128)
            xts = []
            for n in range(NT):
                xt = xp.tile([128, KC, TQ], BF16, tag="xt", name=f"xt_{n}")
                xts.append(xt)
            wq_view = wqkv.rearrange("(kc p) f -> p kc f", p=128)
            if USE_FP8_QK:
                # q/k path: fp8 DoubleRow pairs contraction rows (kc2, e, p)
                x8_view = xT8.rearrange("(kc2 e p) t -> p kc2 e t",
                                        p=128, e=2)
                w8_view = wqk8.rearrange("(kc2 e p) f -> p kc2 e f",
                                         p=128, e=2)
                xt8s = []
                for n in range(NT):
                    xt8 = xp.tile([128, KC2, 2, TQ], FP8, tag="xt8",
                                  name=f"xt8_{n}")
                    xt8s.append(xt8)
                w8_sb = const.tile([128, KC2, 2, 512], FP8, tag="w8")
                for kc2 in range(KC2):
                    nc.sync.dma_start(w8_sb[:, kc2, :, :],
                                      w8_view[:, kc2, :, :])
                    nc.scalar.dma_start(xt8s[0][:, kc2, :, :],
                                        x8_view[:, kc2, :, 0:TQ])
                # cos/sin ahead of the bf16 xt0 chunks: rope needs them by
                # ~8us, while tile-0 v blocks (the xt0 consumers) now run as
                # attn(0) fillers and tolerate a later xt0 arrival
                nc.scalar.dma_start(cos_sb[:], cos_il[:])
                nc.scalar.dma_start(sin_sb[:], sin_il[:])
                w_sb = const.tile([128, KC, 256], BF16, tag="w")
                for kc in range(KC):
                    nc.sync.dma_start(w_sb[:, kc, :],
                                      wq_view[:, kc, 512:768])
                    # alternate queues so xt0 (gating the tile-0 v blocks)
                    # finishes sooner
                    eng = nc.scalar if kc % 2 == 0 else nc.sync
                    eng.dma_start(xts[0][:, kc, :], xt_view[:, kc, 0:TQ])
            else:
                w_sb = const.tile([128, KC, 768], BF16, tag="w")
                nc.scalar.dma_start(cos_sb[:], cos_il[:])
                nc.scalar.dma_start(sin_sb[:], sin_il[:])
                for kc in range(KC):
                    nc.sync.dma_start(w_sb[:, kc, :], wq_view[:, kc, :])
                    nc.scalar.dma_start(xts[0][:, kc, :],
                                        xt_view[:, kc, 0:TQ])

            for n in range(1, NT):
                nc.sync.dma_start(xts[n][:], xt_view[:, :, bass.ts(n, TQ)])
                if USE_FP8_QK:
                    nc.sync.dma_start(xt8s[n][:],
                                      x8_view[:, :, :, bass.ts(n, TQ)])
            wp_sb = const.tile([128, 2, C], BF16, tag="wp")
            wp_view = wp.rearrange("(kb p) f -> p kb f", p=128)
            nc.sync.dma_start(wp_sb[:], wp_view[:])
            if has_battn:
                battn_sb = const.tile([1, 768], BF16, tag="battn")
                nc.scalar.dma_start(battn_sb[:], battn[:])
            if has_bproj:
                bproj_sb = const.tile([1, C], F32R, tag="bproj")
                nc.scalar.dma_start(bproj_sb[:], bproj[:])

            # ---- constants / warmup tiles (no DMA dependency)
            warm_w = const.tile([128, 128], BF16, tag="warm_w")
            nc.vector.memset(warm_w[:], 0.0)
            warm_x = const.tile([128, TQ], BF16, tag="warm_x")
            nc.vector.memset(warm_x[:], 0.0)
            ones_f = const.tile([1, 512], F32, tag="ones_f")
            nc.vector.memset(ones_f[:], 1.0)
            ones_r = const.tile([1, 512], F32R, tag="ones_r")
            nc.vector.tensor_copy(ones_r[:], ones_f[:])
            if has_battn:
                ones_b = const.tile([1, 512], BF16, tag="ones_b")
                nc.vector.tensor_copy(ones_b[:], ones_f[:])

            # HAM warm-up: dense PE activity from t~=0 so the clock gate
            # opens (K=8/8) before the first real matmul, and the PE has
            # work while the input DMAs stream in.
            _warm_ctr = [0]

            def warm_mm():
                wi = _warm_ctr[0]
                _warm_ctr[0] += 1
                pw = ps_qv.tile([128, TQ], F32, tag="pqv", name=f"warm_{wi}")
                nc.tensor.matmul(pw[:], warm_w[:], warm_x[:],
                                 start=True, stop=True)

            for wi in range(N_WARM):
                warm_mm()

            # persistent activations
            # q/k tiles: heads (2g, 2g+1) in rows [0:64],[64:128], lo/hi dims
            # interleaved pairwise within each head
            q_t = [qkp.tile([128, T], BF16, tag=f"q{g}", name=f"q_{g}")
                   for g in range(2)]
            k_t = [qkp.tile([128, T], BF16, tag=f"k{g}", name=f"k_{g}")
                   for g in range(2)]
            v_aug = vaugp.tile([128, 16, HPC * 65], BF16, tag="vaug")
            y0 = yp.tile([128, T], BF16, tag="y0")
            y1 = yp.tile([128, T], BF16, tag="y1")

            # ones columns of v_aug (col 64 of each head's 65-wide slot)
            for tb in range(16):
                va = v_aug[:, tb, :].rearrange("p (h c) -> p h c", c=65)
                nc.vector.memset(va[:, :, 64:65], 1.0)

            # ---------------- qkv projection units -----------------
            def qk_block(n, blk):
                """project q/k block blk (0=q01,1=q23,2=k01,3=k23) of tile n
                and apply rope."""
                tsl = bass.ts(n, TQ)
                xt = xts[n]
                pq = ps_qv.tile([128, TQ], F32, tag="pqv",
                                name=f"pq_{n}_{blk}")
                if USE_FP8_QK:
                    for kc2 in range(KC2):
                        nc.tensor.matmul(
                            pq[:], w8_sb[:, kc2, :, bass.ts(blk, 128)],
                            xt8s[n][:, kc2, :, :],
                            start=(kc2 == 0),
                            stop=(kc2 == KC2 - 1 and not has_battn),
                            perf_mode=DR)
                else:
                    for kc in range(KC):
                        nc.tensor.matmul(
                            pq[:], w_sb[:, kc, bass.ts(blk, 128)],
                            xt[:, kc, :],
                            start=(kc == 0),
                            stop=(kc == KC - 1 and not has_battn))
                if has_battn:
                    nc.tensor.matmul(
                        pq[:], battn_sb[0:1, bass.ts(blk, 128)],
                        ones_b[0:1, :TQ], start=False, stop=True)
                # rope: out = p*cos + swap_pairs(p)*sin_signed
                # PSUM eviction on ACT for early tiles (ACT has slack while
                # DVE is choked by rope math); tile 3's fillers run inside
                # the exp-bound attn(2), so they stay on DVE
                p_s = shp.tile([128, TQ], BF16, tag="ps")
                if n <= 2:
                    nc.scalar.copy(p_s[:], pq[:])
                else:
                    nc.vector.tensor_copy(p_s[:], pq[:])
                sh = shp.tile([128, TQ], BF16, tag="sh")
                nc.vector.stream_shuffle(sh[:], p_s[:], mask=SWAP_EVEN_ODD)
                t1 = tmp.tile([128, TQ], BF16, tag="t")
                nc.vector.tensor_mul(t1[:], p_s[:], cos_sb[:, tsl])
                t2 = tmp.tile([128, TQ], BF16, tag="t")
                nc.vector.tensor_mul(t2[:], sh[:], sin_sb[:, tsl])
                dst = (q_t[0], q_t[1], k_t[0], k_t[1])[blk]
                nc.vector.tensor_add(dst[:, tsl], t1[:], t2[:])

            def v_block(n, tb):
                """project v for token block 4n+tb into v_aug."""
                blk = 4 * n + tb
                xt = xts[n]
                pv = ps_qv.tile([128, TQ], F32, tag="pqv",
                                name=f"pv_{n}_{tb}")
                wv_off = 0 if USE_FP8_QK else 512
                for kc in range(KC):
                    nc.tensor.matmul(
                        pv[:, 0:256], xt[:, kc, bass.ts(tb, 128)],
                        w_sb[:, kc, wv_off:wv_off + 256],
                        start=(kc == 0),
                        stop=(kc == KC - 1 and not has_battn))
                if has_battn:
                    nc.tensor.matmul(
                        pv[:, 0:256], ones_b[0:1, 0:128],
                        battn_sb[0:1, 512:768], start=False, stop=True)
                va = v_aug[:, blk, :].rearrange("p (h c) -> p h c", c=65)
                pv_h = pv[:, 0:256].rearrange("p (h d) -> p h d", d=64)
                if n <= 2:
                    nc.scalar.copy(va[:, :, 0:64], pv_h[:])
                else:
                    nc.vector.tensor_copy(va[:, :, 0:64], pv_h[:])

            def qkv_units(n):
                return ([lambda n=n, b=b_: qk_block(n, b) for b_ in range(4)]
                        + [lambda n=n, t=t_: v_block(n, t) for t_ in range(4)])

            def cproj_unit(i, m, tail=False):
                """c_proj + store for token block m (128 tokens).  In the
                tail (no exps left) half the PSUM evictions go to the ACT
                engine so DVE and ACT drain the last blocks in parallel."""
                msl = bass.ts(m, 128)
                o_t = op.tile([128, C], OUT_DT, tag="o", name=f"o_{m}")
                for n2 in range(2):
                    nsl = bass.ts(n2, 512)
                    po = ps_qv.tile([128, TQ], F32, tag="pqv",
                                    name=f"po_{m}_{n2}")
                    nc.tensor.matmul(po[:], y0[:, msl], wp_sb[:, 0, nsl],
                                     start=True, stop=False)
                    nc.tensor.matmul(po[:], y1[:, msl], wp_sb[:, 1, nsl],
                                     start=False, stop=not has_bproj)
                    if has_bproj:
                        nc.tensor.matmul(po[:], ones_r[0:1, 0:128],
                                         bproj_sb[0:1, nsl],
                                         start=False, stop=True)
                    if tail and n2 == 1:
                        nc.scalar.copy(o_t[:, nsl], po[:])
                    else:
                        nc.vector.tensor_copy(o_t[:, nsl], po[:])
                    if tail:
                        # per-half store so the last transfer starts early
                        nc.sync.dma_start(out[msl, nsl], o_t[:, nsl])
                if not tail:
                    nc.sync.dma_start(out[msl, :], o_t[:])

            def cproj_units(i, tail=False):
                return [lambda i=i, m=m_, tl=tail: cproj_unit(i, m, tl)
                        for m_ in range(4 * i, 4 * i + 4)]

            # ---------------- attention -----------------
            def norm_unit(i, grp, yrs, recs):
                """deferred normalize for BOTH heads of a grp: 1/sumexp is
                broadcast across partitions on the (mostly idle) GPSIMD
                engine, keeping the PE out of the chain entirely."""
                y_tile = y0 if grp == 0 else y1
                for half in range(2):
                    rb = yrp.tile([64, TQ], F32, tag=f"rb{half}")
                    nc.gpsimd.partition_broadcast(rb[:], recs[half][:])
                    y_sl = y_tile[64 * half:64 * half + 64, bass.ts(i, TQ)]
                    nc.vector.tensor_mul(y_sl, yrs[half][:], rb[:])

            def attn_tile(i, fillers, norm_out):
                """attention for query tile i; pops filler units (qkv of
                tile i+1 / cproj of tile i-1 / deferred normalizes) between
                kb iterations.  AV matmuls trail their exp by PIPE
                iterations so the in-order PE queue never waits on the ACT
                engine.  grp-1 normalize units are appended to norm_out.
                Late tiles are exp(ACT)-throughput-bound and the PE runs
                sparse there, which would let the HAM clock gate re-throttle
                it to 1.2GHz -- dummy warm matmuls keep the activity window
                busy."""
                n_k = 4 * i + 4
                warm_every = {0: 2, 1: 3, 2: 3, 3: 4}[i]
                it = 0
                for grp in range(2):
                    kt = k_t[grp]
                    qt = q_t[grp]
                    av = [ps_av.tile([65, TQ], F32, tag="av",
                                     name=f"av_{i}_{grp}_{h}")
                          for h in range(2)]

                    def issue_av(ent, grp=grp, av=av, n_k=n_k):
                        kb, p_t, skip = ent
                        for half in range(2):
                            h = 2 * grp + half
                            nc.tensor.matmul(
                                av[half][:, skip:TQ],
                                v_aug[:, kb, bass.ts(h, 65)],
                                p_t[:, half, skip:TQ],
                                start=(kb == 0),
                                stop=(kb == n_k - 1),
                            )

                    pend = []
                    for kb in range(n_k):
                        ksl = bass.ts(kb, 128)
                        j = kb - 4 * i
                        skip = 128 * j if j > 0 else 0
                        W_v = TQ - skip
                        s2 = ps_s.tile([128, 2 * TQ], F32, tag="s2",
                                       name=f"s_{i}_{grp}_{kb}")
                        for half in range(2):
                            dst = s2[:, half * TQ + skip:(half + 1) * TQ]
                            qsl = bass.ds(i * TQ + skip, W_v)
                            hsl = slice(64 * half, 64 * half + 64)
                            nc.tensor.matmul(dst, kt[hsl, ksl], qt[hsl, qsl],
                                             start=True, stop=True,
                                             tile_position=(64 * half, 0))
                        p_t = pp.tile([128, 2, TQ], BF16, tag="p")
                        s2_v = s2[:].rearrange("p (g t) -> p g t", g=2)
                        nc.scalar.activation(p_t[:, :, skip:TQ],
                                             s2_v[:, :, skip:TQ],
                                             Exp, scale=scale)
                        if j >= 0:
                            # 128-wide causal triangle (keep local col >= row)
                            for half in range(2):
                                nc.gpsimd.affine_select(
                                    out=p_t[:, half, skip:skip + 128],
                                    in_=p_t[:, half, skip:skip + 128],
                                    compare_op=mybir.AluOpType.is_ge,
                                    fill=0.0,
                                    base=0,
                                    pattern=[[1, 128]],
                                    channel_multiplier=-1,
                                )
                        pend.append((kb, p_t, skip))
                        if len(pend) > PIPE:
                            issue_av(pend.pop(0))
                        it += 1
                        # pop fillers whose target iteration has arrived
                        while fillers and fillers[0][0] <= it:
                            _, fn = fillers.pop(0)
                            fn()
                        if warm_every and it % warm_every == 0:
                            warm_mm()
                    for ent in pend:
                        issue_av(ent)

                    # evict the two finished heads; issue the DVE reciprocal
                    # chain now (runs async, batched over both heads), defer
                    # the PE broadcast + y mul so the in-order PE queue
                    # never waits on it here.
                    # per-half base-0 tiles: partition_broadcast reads the
                    # AP's partition 0, so keep each reciprocal in its own
                    # tile rather than a packed one
                    rec_r2 = []
                    for half in range(2):
                        se = rp.tile([1, TQ], F32, tag=f"se{half}")
                        nc.vector.tensor_copy(se[:], av[half][64:65, :])
                        rec = rp.tile([1, TQ], F32, tag=f"r{half}")
                        nc.vector.reciprocal_approx_fast(rec[:], se[:])
                        rec_r2.append(rec)
                    yrs = []
                    for half in range(2):
                        yr = yrp.tile([64, TQ], F32, tag="yr")
                        if grp == 0 and i <= 2:
                            nc.scalar.copy(yr[:], av[half][0:64, :])
                        else:
                            nc.vector.tensor_copy(yr[:], av[half][0:64, :])
                        yrs.append(yr)
                    nu = (lambda i=i, g=grp, ys=yrs, r=rec_r2:
                          norm_unit(i, g, ys, r))
                    if grp == 0:
                        # run during grp 1's kb loop of this tile
                        fillers.append((it + 2, nu))
                        fillers.sort(key=lambda p: p[0])
                    else:
                        norm_out.append(nu)

            # ---------------- schedule -----------------
            # tile 0 q/k (fast fp8 path) runs up front with warm matmuls
            # interleaved; each tile's V projections run as EARLY fillers of
            # its OWN attention tile (AV only consumes v_aug[kb] at
            # iteration kb+PIPE+1, and this keeps the in-order PE queue from
            # blocking on the late-arriving bf16 xt0 DMA).  The next tile's
            # q/k blocks follow (grp0's pair first -- their DVE rope chains
            # must drain before attn(i+1) starts), then cproj(i-1) spreads
            # over the remaining iterations.
            for b_ in (0, 2, 1, 3):
                warm_mm()
                qk_block(0, b_)
            for _ in range(6):
                warm_mm()
            norm_prev = []
            for i in range(NT):
                n_iters = 2 * (4 * i + 4)
                fillers = []
                slot = 1
                for t_ in range(4):
                    fillers.append((slot, lambda n=i, t=t_: v_block(n, t)))
                    slot += 1
                for fn in norm_prev:  # must issue before cproj(i-1)
                    fillers.append((slot, fn))
                    slot += 1
                norm_prev = []
                if i + 1 < NT:
                    for b_ in (0, 2, 1, 3):
                        fillers.append(
                            (slot, lambda n=i + 1, b=b_: qk_block(n, b)))
                        slot += 1
                # cproj fillers go to the LATEST tile their inputs allow:
                # attn(2) and especially attn(3) are exp(ACT)-bound with
                # idle PE capacity, while attn(1)/attn(2) were PE-bound --
                # deferring cproj(0) to attn(2) and cproj(1)+cproj(2) to
                # attn(3) shortens the PE-bound walls for free.
                if i == 2:
                    late = cproj_units(0)
                elif i == 3:
                    late = cproj_units(1) + cproj_units(2)
                else:
                    late = []
                n_l = len(late)
                for li, fn in enumerate(late):
                    s = slot + ((n_iters - slot) * li) // max(n_l, 1)
                    fillers.append((min(s, n_iters), fn))
                fillers.sort(key=lambda p: p[0])
                attn_tile(i, fillers, norm_prev)
                for _, fn in fillers:
                    fn()
            for fn in norm_prev:
                fn()
            for u in cproj_units(NT - 1, tail=True):
                u()
                warm_mm()
                warm_mm()

    nc.finalize()
    return nc


def _get_program(has_battn, has_bproj):
    key = (has_battn, has_bproj)
    if key not in _PROGRAM_CACHE:
        _PROGRAM_CACHE[key] = _build_program(*key)
    return _PROGRAM_CACHE[key]


def _rope_tables_np():
    """cos/sin tables in interleaved-pair layout, sign folded into sin.

    Row 2d   of a 64-block: lo dim d  -> cos(f_d), -sin(f_d)
    Row 2d+1 of a 64-block: hi dim d  -> cos(f_d), +sin(f_d)
    """
    inv_freq = (1.0 / (10000.0 ** (np.arange(0, HD, 2, dtype=np.float32) / HD)))
    t = np.arange(T, dtype=np.float32)
    freqs = np.outer(inv_freq, t).astype(np.float32)      # [32, T]
    cos = np.cos(freqs)
    sin = np.sin(freqs)
    cos64 = np.empty((64, T), dtype=np.float32)
    sin64 = np.empty((64, T), dtype=np.float32)
    cos64[0::2] = cos
    cos64[1::2] = cos
    sin64[0::2] = -sin
    sin64[1::2] = sin
    return (np.ascontiguousarray(np.tile(cos64, (2, 1))),
            np.ascontiguousarray(np.tile(sin64, (2, 1))))


def _install_trace_shim():
    """Optional: lets run_bass_kernel_spmd(trace=True) capture NTFF profiles."""
    import contextlib
    import ctypes
    import types

    so = "/opt/axon/libaxon_pjrt.so"
    if not os.path.exists(so) or "antenv.axon_hooks" in sys.modules:
        return
    try:
        lib = ctypes.CDLL(so)
        if not hasattr(lib, "axon_start_nrt_profile"):
            return
        lib.axon_start_nrt_profile.argtypes = [ctypes.POINTER(ctypes.c_int64),
                                               ctypes.c_size_t]
        lib.axon_start_nrt_profile.restype = ctypes.c_int64
        lib.axon_stop_nrt_profile.argtypes = [ctypes.c_char_p]
        lib.axon_stop_nrt_profile.restype = ctypes.c_int64

        @contextlib.contextmanager
        def _hook(output_dir, device_ids):
            import jax
            jax.devices()
            if device_ids:
                ids = (ctypes.c_int64 * len(device_ids))(*device_ids)
                rc = lib.axon_start_nrt_profile(ids, len(device_ids))
            else:
                rc = lib.axon_start_nrt_profile(None, 0)
            if rc != 0:
                raise RuntimeError(f"axon_start_nrt_profile rc={rc}")
            try:
                yield
            finally:
                n = lib.axon_stop_nrt_profile(str(output_dir).encode())
                print(f"profile: {n} file(s) written to {output_dir}",
                      file=sys.stderr)

        mod = types.ModuleType("antenv.axon_hooks")
        mod.get_axon_ntff_profile_hook = lambda: _hook
        mod.set_axon_ntff_profile_hook = lambda h: None
        sys.modules["antenv.axon_hooks"] = mod
    except Exception:
        pass


def _to_bf16(a):
    import ml_dtypes
    return np.ascontiguousarray(a.astype(ml_dtypes.bfloat16))


def kernel(x, W_attn, b_attn, W_proj, b_proj):
    from concourse.bass_utils import run_bass_kernel_spmd

    x = np.asarray(x, dtype=np.float32)
    W_attn = np.asarray(W_attn, dtype=np.float32)
    b_attn = np.asarray(b_attn, dtype=np.float32)
    W_proj = np.asarray(W_proj, dtype=np.float32)
    b_proj = np.asarray(b_proj, dtype=np.float32)

    has_battn = bool(np.any(b_attn))
    has_bproj = bool(np.any(b_proj))
    nc = _get_program(has_battn, has_bproj)

    cos_il, sin_il = _rope_tables_np()
    dd64 = np.arange(64)
    # interleaved lo/hi order within a head: [0,32,1,33,...,31,63]
    il = np.empty(64, dtype=np.int64)
    il[0::2] = np.arange(32)
    il[1::2] = np.arange(32) + 32

    in_maps = []
    for c in range(N_CORES):
        b = c // 4
        g = c % 4
        hs = 4 * g + np.arange(HPC)
        qcols = (hs[:, None] * HD + il[None, :]).ravel()   # interleaved
        vcols = (hs[:, None] * HD + dd64[None, :]).ravel()  # natural
        cols = np.concatenate([qcols, 1024 + qcols, 2048 + vcols])
        rows = vcols
        m = {
            "xT": _to_bf16(x[b].T),
            "wqkv": _to_bf16(W_attn[:, cols]),
            "cos_il": _to_bf16(cos_il),
            "sin_il": _to_bf16(sin_il),
            "wp": _to_bf16(W_proj[rows, :]),
        }
        if USE_FP8_QK:
            import ml_dtypes
            f8 = ml_dtypes.float8_e4m3fn
            m["xT8"] = np.ascontiguousarray(x[b].T.astype(f8))
            wqk = W_attn[:, cols[:512]] * QK_W_SCALE
            m["wqk8"] = np.ascontiguousarray(wqk.astype(f8))
        if has_battn:
            ba = b_attn[cols].copy()
            if USE_FP8_QK:
                ba[:512] *= QK_W_SCALE
            m["battn"] = _to_bf16(ba[None, :])
        if has_bproj:
            bp = b_proj if g == 0 else np.zeros_like(b_proj)
            m["bproj"] = np.ascontiguousarray(bp[None, :])
        in_maps.append(m)

    trace_dir = os.environ.get("BASSK_TRACE")
    kwargs = {}
    if trace_dir:
        _install_trace_shim()
        kwargs = {"trace": True, "tmpdir": trace_dir,
                  "trace_cores": [0], "stitch_traces": False}

    res = run_bass_kernel_spmd(nc, in_maps, core_ids=list(range(N_CORES)),
                               **kwargs)
    global _LAST_RES
    _LAST_RES = res
    if trace_dir:
        kernel._last_result = res

    out = np.zeros((B, T, C), dtype=np.float32)
    for c in range(N_CORES):
        out[c // 4] += np.asarray(res.results[c]["out"], dtype=np.float32)
    return out
